# revision 35
# baseline (speedup 1.0000x reference)
"""AttentionalPooler Trainium2 kernel (v2).

Full inputs -> full output; batch (8) is data-parallel across the 8
NeuronCores. Per core: LayerNorm(x_b), kv = LN(x_b) @ Wkv, 12-head
cross-attention from 256 pre-computed queries, output projection.

Host-side preprocessing (exact fp32 algebra, batch-independent):
  - q path (LN(query) @ Wq * dh^-0.5, transposed) computed on host.
  - ln_k_w folded into the kv weights; ln_k_b's k-part cancels in
    softmax, its v-part becomes the additive constant r = c_v @ Wout.

v2 design notes (vs v1):
  - attn@v runs in the flipped orientation out[q, head*65] so output
    partitions are full: PSUM-resident accumulators (4 banks, memset +
    start=False accumulate-only matmuls) live across the whole pass --
    no DVE accumulation adds. The ones-column per head provides softmax
    denominators in the same matmuls.
  - The per-head [q,64] blocks are normalized (strided reciprocal +
    per-head tensor_scalar), PE-transposed into pair tiles [128, 256]
    so the output projection contracts K=128 (half the matmul rows).
  - rstd = exp(-0.5*ln(var+eps)) on ACT, batched per quarter, with a
    manual LoadActFuncSet(natural_log_exp_and_others) so the whole
    kernel performs exactly one activation-table load (v1 thrashed
    Sqrt/Exp tables 24x).
  - kT PSUM->SBUF copies go through ACT (Copy), v copies through DVE;
    Pool/GPSIMD cannot access PSUM.
  - attn@v matmuls (N=65, ldweights-bound if adjacent) are paced one-
    or two-at-a-time between the big kv/sim matmuls so their weight
    loads hide under long matmuls.
"""

import sys

sys.path.insert(0, "/opt/trn_rl_repo")

from collections import deque

import numpy as np
import ml_dtypes

import concourse.bass as bass
import concourse.mybir as mybir
import concourse.tile as tile
from concourse import bacc
from concourse.bass_utils import run_bass_kernel_spmd
from concourse.masks import make_identity

F32 = mybir.dt.float32
BF16 = mybir.dt.bfloat16
AX = mybir.AluOpType
AF = mybir.ActivationFunctionType

B = 8
N_TOK = 4096
D_CTX = 1024
D_MODEL = 768
N_HEAD = 12
DH = 64
NQ = 256
INNER = 768
EPS = 1e-5
N_CORES = 8

TOK_TILES = N_TOK // 128  # 32
D_TILES = D_CTX // 128  # 8
E_TILES = INNER // 128  # 6

QS = [2, 2, 4, 4, 4, 6, 6, 4]  # token tiles per quarter
QSTART = [sum(QS[:i]) for i in range(len(QS))]
# natural_log_exp_and_others in act_info.json act_func_sets order
ACT_SET_LN_EXP = 6


def emit_kernel(ctx, tc, out_d, x_d, wp_d, qt_d, wout_d, rrep_d, rep=0):
    nc = tc.nc
    xn_dram = nc.dram_tensor(f"xn_scratch{rep}", [N_TOK, D_CTX], BF16).ap()

    p_const = ctx.enter_context(tc.tile_pool(name="const", bufs=1))
    p_w = ctx.enter_context(tc.tile_pool(name="w", bufs=1))
    p_x = ctx.enter_context(tc.tile_pool(name="x", bufs=8))
    p_stat = ctx.enter_context(tc.tile_pool(name="stat", bufs=7))
    p_xn = ctx.enter_context(tc.tile_pool(name="xn", bufs=4))
    p_xnt = ctx.enter_context(tc.tile_pool(name="xnt", bufs=3))
    p_kt = ctx.enter_context(tc.tile_pool(name="kt", bufs=2))
    p_v = ctx.enter_context(tc.tile_pool(name="v", bufs=2))
    p_attn = ctx.enter_context(tc.tile_pool(name="attn", bufs=6))
    p_tail = ctx.enter_context(tc.tile_pool(name="tail", bufs=1))
    ps_a = ctx.enter_context(tc.tile_pool(name="psa", bufs=2, space="PSUM"))
    ps_sim = ctx.enter_context(tc.tile_pool(name="pssim", bufs=2, space="PSUM"))
    ps_av = ctx.enter_context(tc.tile_pool(name="psav", bufs=4, space="PSUM"))

    # --- constants, one activation-table load for the whole kernel -------
    warm = p_const.tile([128, 256], BF16, tag="warm")
    nc.vector.memset(warm[:], 1.0)
    eps_t = p_const.tile([128, 1], F32, tag="eps")
    nc.vector.memset(eps_t[:], EPS)
    ident = p_const.tile([128, 128], BF16, tag="ident")
    make_identity(nc, ident[:])
    nc.scalar.add_instruction(
        mybir.InstLoadActFuncSet(
            name=nc.get_next_instruction_name(),
            act_func_set_id=ACT_SET_LN_EXP,
            ins=[],
            outs=[],
        )
    )

    # PE warm-up block (no data deps): keeps the HAM clock gate fed from
    # t=0 while the LN pipeline fills
    def warm_block(n, tag):
        for w in range(n):
            wps = ps_sim.tile([128, 2, NQ], F32, tag="s", name=f"warm{tag}_{w}")
            for _ in range(2):
                nc.tensor.matmul(
                    out=wps.rearrange("p a b -> p (a b)")[:, 0:256],
                    lhsT=warm[:, 0:128],
                    rhs=warm[:],
                    start=True,
                    stop=True,
                )

    warm_block(8, "a")

    # held attn@v accumulators: [q-tile, half] -> [128, 6 heads, 64+1]
    pav = {
        (qt, hf): ps_av.tile([128, 6, DH + 1], F32, tag="av",
                             name=f"pav{qt}_{hf}")
        for qt in range(2)
        for hf in range(2)
    }

    # --- per-tile LN prep (pipelined) ------------------------------------
    xn_tiles = {}
    x_tiles = {}

    def issue_x(i):
        xt = p_x.tile([128, D_CTX], BF16, tag="x", name=f"x{i}")
        x_tiles[i] = xt
        # x loads ride the SWDGE (Pool) queue so the HWDGE queue stays
        # dedicated to xn stores + transposes (their issue rate gates the
        # quarter pipeline)
        nc.gpsimd.dma_start(out=xt[:], in_=x_d[i * 128 : (i + 1) * 128, :])

    def prep_tile(i):
        xt = x_tiles.pop(i)
        st = p_stat.tile([128, 2, 6], F32, tag="st", name=f"st{i}")
        nc.vector.bn_stats(out=st[:, 0, :], in_=xt[:, 0:512])
        nc.vector.bn_stats(out=st[:, 1, :], in_=xt[:, 512:1024])
        mv = p_stat.tile([128, 2], F32, tag="mv", name=f"mv{i}")
        nc.vector.bn_aggr(out=mv[:], in_=st[:])
        # rstd = exp(-0.5 * ln(var + eps)): stays on the exp table
        lnv = p_stat.tile([128, 1], F32, tag="lnv", name=f"lnv{i}")
        nc.scalar.activation(
            out=lnv[:], in_=mv[:, 1:2], func=AF.Ln, bias=eps_t[:], scale=1.0
        )
        rstd = p_stat.tile([128, 1], F32, tag="rstd", name=f"rstd{i}")
        nc.scalar.activation(out=rstd[:], in_=lnv[:], func=AF.Exp, scale=-0.5)
        negmr = p_stat.tile([128, 1], F32, tag="negmr", name=f"negmr{i}")
        nc.vector.scalar_tensor_tensor(
            out=negmr[:],
            in0=mv[:, 0:1],
            scalar=-1.0,
            in1=rstd[:],
            op0=AX.mult,
            op1=AX.mult,
        )
        xn = p_xn.tile([128, D_CTX], BF16, tag="xn", name=f"xn{i}")
        xn_tiles[i] = xn
        nc.vector.tensor_scalar(
            out=xn[:],
            in0=xt[:],
            scalar1=rstd[:, 0:1],
            scalar2=negmr[:, 0:1],
            op0=AX.mult,
            op1=AX.add,
        )
        if i >= 2:
            # quarter 0 is transposed on-chip; only later tiles bounce
            # through DRAM for the xbar transpose
            nc.sync.dma_start(out=xn_dram[i * 128 : (i + 1) * 128, :], in_=xn[:])
        if 2 <= i < 8:
            # paced HAM warm-up chained off this tile's data
            wps = ps_sim.tile([128, 2, NQ], F32, tag="s", name=f"wp{i}")
            for _ in range(4):
                nc.tensor.matmul(
                    out=wps.rearrange("p a b -> p (a b)")[:, 0:256],
                    lhsT=xn[:, 0:128],
                    rhs=xn[:, 0:256],
                    start=True,
                    stop=True,
                )

    # --- paced attn@v emission -------------------------------------------
    pending = deque()

    def pace(n=1):
        for _ in range(n):
            if pending:
                pending.popleft()()

    def emit_transposes(q):
        j0, nj = QSTART[q], QS[q]
        xnt = p_xnt.tile([128, D_TILES, 768], BF16, tag="xnt", name=f"xnt{q}")
        for d in range(D_TILES):
            nc.sync.dma_start(
                out=xnt[:, d, 0 : nj * 128],
                in_=xn_dram[j0 * 128 : (j0 + nj) * 128, d * 128 : (d + 1) * 128],
                transpose=True,
            )
        return xnt

    # --- get the pipeline going ------------------------------------------
    # DMA transfers serialize in issue order, so the sync queue sequences
    # the early window by need time: k-weights + queries first, then the
    # quarter-0 stores/transposes, then the v-half weights.
    wp = p_w.tile([128, D_TILES, 2 * INNER], BF16, tag="wp")
    wp_r = wp_d.rearrange("(t p) n -> p t n", p=128)
    for d2 in range(0, D_TILES, 2):
        # chunked so the serial DMA resource interleaves the x loads
        nc.sync.dma_start(
            out=wp[:, d2 : d2 + 2, 0:INNER], in_=wp_r[:, d2 : d2 + 2, 0:INNER]
        )
    qt_sb = p_w.tile([128, E_TILES, NQ], BF16, tag="qt")
    nc.sync.dma_start(out=qt_sb[:], in_=qt_d.rearrange("(t p) n -> p t n", p=128))

    for i in range(6):
        issue_x(i)
    prep_tile(0)
    prep_tile(1)

    # quarter 0's xnt is built with PE transposes straight from SBUF --
    # no DRAM bounce, so PE gets real work ~4us in and the early DMA
    # window stays free for quarter 1+. The PSUM->SBUF copies are
    # interleaved with the later prep chains so they never head-of-line
    # block the Ln/Exp rstd ops on ACT or the stats on DVE.
    xnt = p_xnt.tile([128, D_TILES, 768], BF16, tag="xnt", name="xnt0")
    for j in range(QS[0]):
        for d in range(D_TILES):
            pt0 = ps_sim.tile([128, 128], BF16, tag="s", name=f"pt0_{j}_{d}")
            nc.tensor.transpose(
                pt0[:], xn_tiles[j][:, d * 128 : (d + 1) * 128], ident[:]
            )
            if (j + d) % 2 == 0:
                nc.scalar.activation(
                    out=xnt[:, d, j * 128 : (j + 1) * 128], in_=pt0[:],
                    func=AF.Copy,
                )
            else:
                nc.vector.tensor_copy(
                    out=xnt[:, d, j * 128 : (j + 1) * 128], in_=pt0[:]
                )
    warm_block(6, "b")

    for i in range(2, 6):
        prep_tile(i)
    next_xnt = emit_transposes(1)

    for d2 in range(0, D_TILES, 2):
        nc.sync.dma_start(
            out=wp[:, d2 : d2 + 2, INNER : 2 * INNER],
            in_=wp_r[:, d2 : d2 + 2, INNER : 2 * INNER],
        )
    wout_sb = p_w.tile([128, E_TILES, D_MODEL], BF16, tag="wout")
    rrep = p_w.tile([128, D_MODEL], F32, tag="rrep")

    for t in pav.values():
        nc.vector.memset(t[:], 0.0)

    # --- main pass --------------------------------------------------------
    prepped = 6
    for q, (j0, nj) in enumerate(zip(QSTART, QS)):
        ncol = nj * 128

        if q == 2:
            nc.gpsimd.dma_start(
                out=wout_sb[:], in_=wout_d.rearrange("(t p) n -> p t n", p=128)
            )
            nc.gpsimd.dma_start(out=rrep[:], in_=rrep_d[:])

        # x loads for quarter q+2 issue now (DMA lead time); their LN chain
        # runs at the end of this quarter so the stats/rstd ops never
        # head-of-line block the ACT queue on a late x transfer
        prep_goal = QSTART[q + 2] + QS[q + 2] if q + 2 < len(QS) else TOK_TILES
        for i in range(prepped, prep_goal):
            issue_x(i)

        # v projection first (except quarter 0, whose v-half weights are
        # still in flight): gives ACT room to drain prior exps and gives
        # the paced attn@v matmuls long matmuls to hide under
        vt = p_v.tile([128, 6, N_HEAD, DH + 1], BF16, tag="v", name=f"v{q}")
        nc.vector.memset(vt[:, 0:nj, :, DH : DH + 1], 1.0)

        def v_chunks(vt=vt, q=q, nj=nj, xnt=xnt):
            for jj in range(nj):
                for h6 in range(2):
                    ps = ps_a.tile([128, 512], F32, tag="a",
                                   name=f"pv{q}_{jj}_{h6}")
                    for d in range(D_TILES):
                        nc.tensor.matmul(
                            out=ps[:, 0:384],
                            lhsT=xnt[:, d, jj * 128 : (jj + 1) * 128],
                            rhs=wp[:, d,
                                   INNER + h6 * 384 : INNER + (h6 + 1) * 384],
                            start=(d == 0),
                            stop=(d == D_TILES - 1),
                        )
                        pace(1)
                    nc.vector.tensor_copy(
                        out=vt[:, jj, h6 * 6 : (h6 + 1) * 6, 0:DH],
                        in_=ps[:, 0:384].rearrange("p (h dh) -> p h dh", dh=DH),
                    )

        # at q==0 the v-half weights are still in flight; v is emitted mid-
        # pair-loop and this quarter's attn@v thunks are held until then so
        # their emission follows the v-copy emission
        hold = [] if q == 0 else None
        if q > 0:
            v_chunks()

        # per pair: kT chunks then sim+exp, attn@v paced throughout
        kt = p_kt.tile([128, E_TILES, 768], BF16, tag="kt", name=f"kt{q}")
        for p in range(E_TILES):
            e = p
            for c0 in range(0, ncol, 512):
                cw = min(512, ncol - c0)
                ps = ps_a.tile([128, 512], F32, tag="a", name=f"pk{q}_{e}_{c0}")
                for d in range(D_TILES):
                    nc.tensor.matmul(
                        out=ps[:, 0:cw],
                        lhsT=wp[:, d, e * 128 : (e + 1) * 128],
                        rhs=xnt[:, d, c0 : c0 + cw],
                        start=(d == 0),
                        stop=(d == D_TILES - 1),
                    )
                    pace(1)
                nc.scalar.activation(
                    out=kt[:, e, c0 : c0 + cw], in_=ps[:, 0:cw], func=AF.Copy
                )
            if q == 0 and p == 2:
                # v-half weights have landed by now; emit quarter-0 v
                # before the attn@v matmuls of early pairs need it
                v_chunks()
                pending.extend(hold)
                hold = None
            attn_t = {}
            for hh in range(2):
                attn_t[hh] = p_attn.tile(
                    [128, 6, NQ], BF16, tag="attn", bufs=6, name=f"at{q}_{p}_{hh}"
                )
            for g0 in range(0, nj, 2):
                for hh in range(2):
                    base = 64 * hh
                    ps = ps_sim.tile(
                        [128, 2, NQ], F32, tag="s", name=f"psim{q}_{p}_{g0}_{hh}"
                    )
                    for j2 in range(2):
                        nc.tensor.matmul(
                            out=ps[:, j2, :],
                            lhsT=kt[base : base + 64, p,
                                    (g0 + j2) * 128 : (g0 + j2 + 1) * 128],
                            rhs=qt_sb[base : base + 64, p, :],
                            start=True,
                            stop=True,
                        )
                        pace(2)
                    nc.scalar.activation(
                        out=attn_t[hh][:, g0 : g0 + 2, :], in_=ps[:], func=AF.Exp
                    )
            # queue this pair's attn@v matmuls; they are paced into the
            # following big matmuls so their ldweights stay hidden
            for hh in range(2):
                h = 2 * p + hh
                hf, col = h // 6, h % 6
                at = attn_t[hh]
                for qt2 in range(2):
                    for jj in range(nj):
                        def av_mm(h=h, hf=hf, col=col, qt2=qt2, jj=jj,
                                  at=at, vt=vt):
                            nc.tensor.matmul(
                                out=pav[(qt2, hf)][:, col, :],
                                lhsT=at[:, jj, qt2 * 128 : (qt2 + 1) * 128],
                                rhs=vt[:, jj, h, :],
                                start=False,
                                stop=True,
                                skip_group_check=True,
                            )
                        (pending if hold is None else hold).append(av_mm)

        # LN chains + stores for quarter q+2's tiles, then its transposes
        while prepped < prep_goal:
            prep_tile(prepped)
            prepped += 1
        next_xnt2 = emit_transposes(q + 2) if q + 2 < len(QS) else None

        xnt, next_xnt = next_xnt, next_xnt2

    while pending:
        pending.popleft()()

    # --- tail: normalize, transpose, output projection --------------------
    ot = {}
    for qt2 in range(2):
        ot[qt2] = p_tail.tile([128, N_HEAD, DH], BF16, tag="ot", bufs=2,
                              name=f"ot{qt2}")
        for hf in range(2):
            rec = p_tail.tile([128, 6], F32, tag="rec", bufs=4,
                              name=f"rec{qt2}_{hf}")
            nc.vector.reciprocal(out=rec[:], in_=pav[(qt2, hf)][:, :, DH : DH + 1])
            for c in range(6):
                h = hf * 6 + c
                nc.vector.tensor_scalar_mul(
                    out=ot[qt2][:, h, :],
                    in0=pav[(qt2, hf)][:, c, 0:DH],
                    scalar1=rec[:, c : c + 1],
                )

    ott = {}
    for kt2 in range(E_TILES):
        ott[kt2] = p_tail.tile([128, NQ], BF16, tag="ott", bufs=6,
                               name=f"ott{kt2}")
        for qt2 in range(2):
            pt = ps_sim.tile([128, 128], BF16, tag="s", name=f"pt{kt2}_{qt2}")
            nc.tensor.transpose(
                pt[:], ot[qt2][:, 2 * kt2 : 2 * kt2 + 2, :], ident[:]
            )
            nc.scalar.activation(
                out=ott[kt2][:, qt2 * 128 : (qt2 + 1) * 128],
                in_=pt[:],
                func=AF.Copy,
            )

    for qt2 in range(2):
        fin = p_tail.tile([128, D_MODEL], F32, tag="fin", bufs=2,
                          name=f"fin{qt2}")
        for n2 in range(2):
            ps = ps_a.tile([128, 512], F32, tag="a", name=f"pf{qt2}_{n2}")
            for kt2 in range(E_TILES):
                nc.tensor.matmul(
                    out=ps[:, 0:384],
                    lhsT=ott[kt2][:, qt2 * 128 : (qt2 + 1) * 128],
                    rhs=wout_sb[:, kt2, n2 * 384 : (n2 + 1) * 384],
                    start=(kt2 == 0),
                    stop=(kt2 == E_TILES - 1),
                )
            nc.vector.tensor_tensor(
                out=fin[:, n2 * 384 : (n2 + 1) * 384],
                in0=ps[:, 0:384],
                in1=rrep[:, n2 * 384 : (n2 + 1) * 384],
                op=AX.add,
            )
        nc.sync.dma_start(
            out=out_d[qt2 * 128 : (qt2 + 1) * 128, :], in_=fin[:]
        )


def build_nc(reps=1):
    nc = bacc.Bacc(
        "TRN2", target_bir_lowering=False, debug=False, num_devices=N_CORES
    )
    x_d = nc.dram_tensor("x", [N_TOK, D_CTX], BF16, kind="ExternalInput").ap()
    wp_d = nc.dram_tensor("wp", [D_CTX, 2 * INNER], BF16, kind="ExternalInput").ap()
    qt_d = nc.dram_tensor("qt", [INNER, NQ], BF16, kind="ExternalInput").ap()
    wout_d = nc.dram_tensor(
        "wout", [INNER, D_MODEL], BF16, kind="ExternalInput"
    ).ap()
    rrep_d = nc.dram_tensor("rrep", [128, D_MODEL], F32, kind="ExternalInput").ap()
    out_d = nc.dram_tensor("out", [NQ, D_MODEL], F32, kind="ExternalOutput").ap()
    from contextlib import ExitStack

    with tile.TileContext(nc) as tc:
        for rep in range(reps):
            with ExitStack() as ctx:
                emit_kernel(ctx, tc, out_d, x_d, wp_d, qt_d, wout_d, rrep_d, rep=rep)
    nc.compile()
    return nc


def host_prep(query, ln_q_w, ln_q_b, ln_k_w, ln_k_b, Wq, Wkv, Wout):
    """Batch-independent fp32 preprocessing. Returns per-core input dict
    (minus x)."""
    query = np.asarray(query, np.float32)
    mu = query.mean(-1, keepdims=True)
    var = ((query - mu) ** 2).mean(-1, keepdims=True)
    qn = (query - mu) / np.sqrt(var + EPS) * ln_q_w + ln_q_b
    qmat = (qn @ np.asarray(Wq, np.float32)) * (DH**-0.5)  # [NQ, INNER]
    qT = np.ascontiguousarray(qmat.T).astype(ml_dtypes.bfloat16)

    Wkv = np.asarray(Wkv, np.float32)
    Wp = (np.asarray(ln_k_w, np.float32)[:, None] * Wkv).astype(ml_dtypes.bfloat16)
    c = np.asarray(ln_k_b, np.float32) @ Wkv  # [2*INNER]
    c_v = c[INNER:]
    Wout = np.asarray(Wout, np.float32)
    r = c_v @ Wout  # [D_MODEL]
    rrep = np.ascontiguousarray(np.broadcast_to(r, (128, D_MODEL))).astype(np.float32)
    wout_arr = np.ascontiguousarray(Wout).astype(ml_dtypes.bfloat16)
    return {"wp": Wp, "qt": qT, "wout": wout_arr, "rrep": rrep}


_NC_CACHE = {}


def get_nc():
    if "nc" not in _NC_CACHE:
        _NC_CACHE["nc"] = build_nc()
    return _NC_CACHE["nc"]


def kernel(x, query, ln_q_w, ln_q_b, ln_k_w, ln_k_b, Wq, Wkv, Wout):
    x = np.asarray(x, np.float32)
    shared = host_prep(query, ln_q_w, ln_q_b, ln_k_w, ln_k_b, Wq, Wkv, Wout)
    in_maps = [
        {"x": np.ascontiguousarray(x[b]).astype(ml_dtypes.bfloat16), **shared}
        for b in range(B)
    ]
    nc = get_nc()
    res = run_bass_kernel_spmd(nc, in_maps, list(range(N_CORES)))
    return np.stack([res.results[b]["out"] for b in range(B)], axis=0)
